# revision 18
# baseline (speedup 1.0000x reference)
"""Trainium2 Bass kernel for nn_Loss_9749575762182.

Computes two scalar losses over (8192, 2048) fp32 tensors:
  wmse = mean((weight[:,None] * (target - input))**2)
  wcl  = mean(|(st*ln(tp+eps) + (1-st)*ln(1-tp+eps)) * obrT|)

Strategy: data-parallel over the row axis across 8 NeuronCores
(1024 rows each). Each core streams its 5 x 8MB tensor slices through
SBUF in eight [128, 2048] tiles, producing per-partition partial sums;
the tiny [128, 20] partials land back in DRAM and the host finishes
the reduction in float64.

The kernel is HBM-bound: 40MB/core over ~358GB/s peak = 112us floor;
~330GB/s (121us) is the observed practical stream rate. Evolution:
  v1 155.7us: 8 tiles, coarse tail (a full-2048 serial chain after the
      last byte) cost ~24us past the end of streaming.
  v2 143.1us: 18 column-split chunks (1024/512 wide). Short tail, but
      2-4KB DMA descriptors + 2x the dispatches/semaphores slowed the
      stream by ~4us and added ~1us stalls at chunk seams.
  v3: loads stay full [128, 2048] tiles (1MB contiguous, 8KB descriptor
      rows = best DMA efficiency, 40 dispatches total), while COMPUTE
      on the last tile is split into 1024+512+512 column passes so the
      post-stream serial chain is ~512-wide (~4us). Other wins kept:
      - First ACT instruction is an Ln touch: Bacc's
        insert_act_table_loads picks act-func-set 5 (ln+square+abs+copy)
        once, instead of set 0 + a 1283ns reload at the first real Ln.
      - mse/cl partials accumulate into ONE [128, 20] tile; the final
        store is issued from the ACT engine right after the last accum
        (ACT program order: no cross-engine semaphore, HWDGE latency).
      - Loads issue q (target_pre) first so the ACT ln chain starts
        as early as possible; per-tile compute runs l1/l2 first.

Per tile the math keeps the Vector and Scalar engines well under the
~14.8us/tile DMA budget:
  ACT: l1 = Ln(tp + eps)          (bias/scale fold the affine into the LUT)
  ACT: l2 = Ln(-tp + (1+eps))
  DVE: diff = target - input                     (tensor_tensor sub)
  ACT: Square(diff * w)  + accum -> mse partial  (scale = per-partition w)
  DVE: d = l1 - l2 ; m = st * d ; b = m + l2 ; po = b * obrT
  ACT: Abs(po) + accum -> cl partial

Hard-won environment notes (axon-tunneled trn2, this toolchain):
  - Build on bacc.Bacc() and call nc.finalize() before run_bass_via_pjrt;
    raw bass.Bass() BIR fails walrus ("Reg has not been allocated"), and
    without Bacc's generate_event_semaphores pass any instruction with
    >1 semaphore wait dies in codegen ("Too many sync wait commands").
  - tensor_tensor_reduce compiles + simulates fine but faults on real HW
    via the PJRT path; ACT Abs with accum_out replaces it.
  - Big loads go through nc.sync.dma_start (HW-DGE, fans out across HW
    queues): all-gpsimd SWDGE funnels through ONE dynamic queue
    (~216 GB/s ceiling observed -> 185us); HW-DGE gets 153us.
  - The CoreV3 ISA allows one sync-wait per instruction, and Tile
    doesn't split excess waits for free. Discipline: every instruction
    may depend on at most ONE foreign semaphore; tiny "touch" ops
    consume extra waits so the real consumers inherit them via engine
    program order / already-observed clocks.
"""

import os
import sys

if "/opt/trn_rl_repo" not in sys.path:
    sys.path.insert(0, "/opt/trn_rl_repo")

import numpy as np

N, D = 8192, 2048
NCORES = 8
ROWS = N // NCORES  # rows per core
P = 128             # SBUF partitions
NT = ROWS // P      # row-blocks per core (8)
EPS = 1e-10

# accumulator columns: 9 mse (7 full tiles via ACT Square + 2 DVE tail
# halves), 16 cl (two per tile: sum(c1*l1), sum(c2*l2)).
MSE_COLS = NT + 1
CL_COLS = 2 * NT
NCOLS = MSE_COLS + CL_COLS

_CACHE = {}


def build(rows=ROWS, d=D):
    import concourse.bacc as bacc
    import concourse.tile as tile
    from concourse import mybir

    f32 = mybir.dt.float32
    bf16 = mybir.dt.bfloat16
    ACTF = mybir.ActivationFunctionType
    ALU = mybir.AluOpType

    nc = bacc.Bacc()
    inp = nc.dram_tensor("input", [rows, d], f32, kind="ExternalInput")
    tgt = nc.dram_tensor("target", [rows, d], f32, kind="ExternalInput")
    wgt = nc.dram_tensor("weight", [rows], f32, kind="ExternalInput")
    st = nc.dram_tensor("sub_target", [rows, d], f32, kind="ExternalInput")
    tp = nc.dram_tensor("target_pre", [rows, d], f32, kind="ExternalInput")
    ob = nc.dram_tensor("sub_obrT", [rows, d], f32, kind="ExternalInput")
    # host-transposed weight columns: wcols[p, t] = w[t*128 + p], plus the
    # square. Contiguous [128, NT] HWDGE loads replace a 1024-descriptor
    # SWDGE gather that serialized ~17us onto one DMA queue (the stream
    # straggler in every earlier version of this kernel).
    wcols_d = nc.dram_tensor("wcols", [P, NT], f32, kind="ExternalInput")
    w2cols_d = nc.dram_tensor("w2cols", [P, NT], f32, kind="ExternalInput")
    out = nc.dram_tensor("partials", [P, NCOLS], f32, kind="ExternalOutput")

    inp_t = inp.rearrange("(t p) d -> t p d", p=P)
    tgt_t = tgt.rearrange("(t p) d -> t p d", p=P)
    st_t = st.rearrange("(t p) d -> t p d", p=P)
    tp_t = tp.rearrange("(t p) d -> t p d", p=P)
    ob_t = ob.rearrange("(t p) d -> t p d", p=P)

    MM = 512  # PE max moving free-dim

    with tile.TileContext(nc) as tc:
        with (
            tc.tile_pool(name="singles", bufs=1) as singles,
            tc.tile_pool(name="in_p", bufs=3) as in_p,
            tc.tile_pool(name="tgt_p", bufs=3) as tgt_p,
            tc.tile_pool(name="st_p", bufs=4) as st_p,
            tc.tile_pool(name="tp_p", bufs=3) as tp_p,
            tc.tile_pool(name="ob_p", bufs=4) as ob_p,
            tc.tile_pool(name="l1_p", bufs=2) as l1_p,
            tc.tile_pool(name="l2_p", bufs=2) as l2_p,
            tc.tile_pool(name="d_p", bufs=2) as d_p,
            tc.tile_pool(name="c1_p", bufs=2) as c1_p,
            tc.tile_pool(name="c2_p", bufs=2) as c2_p,
            tc.tile_pool(name="trash_p", bufs=1) as trash_p,
            tc.tile_pool(name="sqd_p", bufs=1) as sqd_p,
            tc.psum_pool(name="ps_p", bufs=2) as ps_p,
        ):
            w_cols = singles.tile([P, NT], f32)
            nc.scalar.dma_start(out=w_cols, in_=wcols_d[:, 0:NT])
            w2 = singles.tile([P, NT], f32)
            nc.scalar.dma_start(out=w2, in_=w2cols_d[:, 0:NT])
            # identity / -identity stationaries for the PE diff trick:
            # PSUM = I @ g + (-I) @ x  ==  g - x, exact in fp32.
            ident = singles.tile([P, P], f32)
            nc.gpsimd.memset(ident, 1.0)
            nc.gpsimd.affine_select(
                out=ident, in_=ident, compare_op=ALU.is_equal, fill=0.0,
                base=0, pattern=[[-1, P]], channel_multiplier=1,
            )
            nident = singles.tile([P, P], f32)
            nc.gpsimd.memset(nident, -1.0)
            nc.gpsimd.affine_select(
                out=nident, in_=nident, compare_op=ALU.is_equal, fill=0.0,
                base=0, pattern=[[-1, P]], channel_multiplier=1,
            )

            cols = singles.tile([P, NCOLS], f32)
            eps_b = singles.tile([P, 1], f32)
            nc.vector.memset(eps_b, EPS)
            one_eps_b = singles.tile([P, 1], f32)
            nc.vector.memset(one_eps_b, 1.0 + EPS)
            zero_b = singles.tile([P, 1], f32)
            nc.vector.memset(zero_b, 0.0)

            gate_b = singles.tile([P, 1], f32)
            touch_d = singles.tile([P, 1], f32)
            atouch_d = singles.tile([P, 1], f32)
            # First ACT instruction is an Ln: loads act-func-set 5 once and
            # consumes the DVE-memset wait; the next two consume the
            # wcols/w2 DMA-completion waits.
            nc.scalar.activation(
                out=atouch_d, in_=zero_b, func=ACTF.Ln, bias=zero_b, scale=1.0
            )
            nc.scalar.activation(
                out=atouch_d, in_=w_cols[:, 0:1], func=ACTF.Ln, bias=zero_b, scale=1.0
            )
            nc.scalar.activation(
                out=atouch_d, in_=w2[:, 0:1], func=ACTF.Ln, bias=zero_b, scale=1.0
            )

            # sq(t) is deferred one tile and pinned after the next tile's
            # Lns via the gate bias (a zero that data-depends on l2), so
            # DVE's cl-accums never queue behind it on ACT.
            pending_sq = None

            def emit_sq(p, l2gate):
                ddp, wc, col = p
                nc.scalar.activation(
                    out=gate_b, in_=l2gate[:, 0:1], func=ACTF.Copy, scale=0.0
                )
                sq = sqd_p.tile([P, d], bf16, name="sq")
                nc.scalar.activation(
                    out=sq, in_=ddp, func=ACTF.Square, bias=gate_b,
                    scale=wc, accum_out=col,
                )  # waits PE (dd in PSUM)

            # ---- tiles 0..6: loads q,s,o,x,g ([128,2048], 8KB descriptor
            # rows = line-rate DMA). PE computes dd = g - x into PSUM; ACT:
            # l1, l2, Square(prev); DVE: cc1, cc2 and the two cl accums.
            for t in range(NT - 1):
                q = tp_p.tile([P, d], f32, name="q")
                nc.sync.dma_start(out=q, in_=tp_t[t])
                s = st_p.tile([P, d], f32, name="s")
                nc.sync.dma_start(out=s, in_=st_t[t])
                o = ob_p.tile([P, d], f32, name="o")
                nc.sync.dma_start(out=o, in_=ob_t[t])
                x = in_p.tile([P, d], f32, name="x")
                nc.sync.dma_start(out=x, in_=inp_t[t])
                g = tgt_p.tile([P, d], f32, name="g")
                nc.sync.dma_start(out=g, in_=tgt_t[t])

                l1 = l1_p.tile([P, d], bf16, name="l1")
                nc.scalar.activation(out=l1, in_=q, func=ACTF.Ln, bias=eps_b, scale=1.0)
                l2 = l2_p.tile([P, d], bf16, name="l2")
                nc.scalar.activation(
                    out=l2, in_=q, func=ACTF.Ln, bias=one_eps_b, scale=-1.0
                )
                if pending_sq is not None:
                    emit_sq(pending_sq, l2)

                dd_ps = ps_p.tile([P, d], f32, name="dd_ps")
                for m in range(d // MM):
                    msl = slice(m * MM, (m + 1) * MM)
                    nc.tensor.matmul(
                        out=dd_ps[:, msl], lhsT=ident, rhs=g[:, msl],
                        start=True, stop=False,
                    )
                    nc.tensor.matmul(
                        out=dd_ps[:, msl], lhsT=nident, rhs=x[:, msl],
                        start=False, stop=True,
                    )
                pending_sq = (dd_ps, w_cols[:, t : t + 1], cols[:, t : t + 1])

                nc.vector.tensor_copy(touch_d, s[:, 0:1])  # consume s-DMA wait
                cc1 = c1_p.tile([P, d], bf16, name="cc1")
                nc.vector.scalar_tensor_tensor(
                    cc1, s, 0.0, o, ALU.bypass, ALU.mult
                )  # waits o-DMA
                cc2 = c2_p.tile([P, d], bf16, name="cc2")
                nc.vector.scalar_tensor_tensor(cc2, o, 0.0, cc1, ALU.bypass, ALU.subtract)
                nc.vector.tensor_copy(touch_d, l2[:, 0:1])  # consume ACT-l2 wait
                tr = trash_p.tile([P, d], bf16, name="tr")
                nc.vector.scalar_tensor_tensor(
                    tr, cc1, 0.0, l1, ALU.bypass, ALU.mult,
                    accum_out=cols[:, MSE_COLS + 2 * t : MSE_COLS + 2 * t + 1],
                )
                tr2 = trash_p.tile([P, d], bf16, name="tr")
                nc.vector.scalar_tensor_tensor(
                    tr2, cc2, 0.0, l2, ALU.bypass, ALU.mult,
                    accum_out=cols[:, MSE_COLS + 2 * t + 1 : MSE_COLS + 2 * t + 2],
                )

            # ---- tile 7: loads s,o,q then x,g in halves; compute all-DVE
            # (dd via STT with w^2) so the post-stream chain is just
            # dd1 + mse1 + the ACT-issued store.
            t = NT - 1
            s = st_p.tile([P, d], f32, name="s")
            nc.sync.dma_start(out=s, in_=st_t[t])
            o = ob_p.tile([P, d], f32, name="o")
            nc.sync.dma_start(out=o, in_=ob_t[t])
            q = tp_p.tile([P, d], f32, name="q")
            nc.sync.dma_start(out=q, in_=tp_t[t])
            xh, gh = [], []
            for h in range(2):
                c0, c1 = h * (d // 2), (h + 1) * (d // 2)
                xk = in_p.tile([P, d // 2], f32, name="x")
                nc.sync.dma_start(out=xk, in_=inp_t[t][:, c0:c1])
                gk = tgt_p.tile([P, d // 2], f32, name="g")
                nc.sync.dma_start(out=gk, in_=tgt_t[t][:, c0:c1])
                xh.append(xk)
                gh.append(gk)

            l1 = l1_p.tile([P, d], bf16, name="l1")
            nc.scalar.activation(out=l1, in_=q, func=ACTF.Ln, bias=eps_b, scale=1.0)
            l2 = l2_p.tile([P, d], bf16, name="l2")
            nc.scalar.activation(
                out=l2, in_=q, func=ACTF.Ln, bias=one_eps_b, scale=-1.0
            )
            emit_sq(pending_sq, l2)  # sq(6), after l2(7) in ACT order

            nc.vector.tensor_copy(touch_d, s[:, 0:1])  # consume s-DMA wait
            cc1 = c1_p.tile([P, d], bf16, name="cc1")
            nc.vector.scalar_tensor_tensor(
                cc1, s, 0.0, o, ALU.bypass, ALU.mult
            )  # waits o-DMA
            cc2 = c2_p.tile([P, d], bf16, name="cc2")
            nc.vector.scalar_tensor_tensor(cc2, o, 0.0, cc1, ALU.bypass, ALU.subtract)
            nc.vector.tensor_copy(touch_d, l2[:, 0:1])  # consume ACT-l2 wait
            tr = trash_p.tile([P, d], bf16, name="tr")
            nc.vector.scalar_tensor_tensor(
                tr, cc1, 0.0, l1, ALU.bypass, ALU.mult,
                accum_out=cols[:, MSE_COLS + 2 * t : MSE_COLS + 2 * t + 1],
            )
            tr2 = trash_p.tile([P, d], bf16, name="tr")
            nc.vector.scalar_tensor_tensor(
                tr2, cc2, 0.0, l2, ALU.bypass, ALU.mult,
                accum_out=cols[:, MSE_COLS + 2 * t + 1 : MSE_COLS + 2 * t + 2],
            )
            for h in range(2):
                nc.vector.tensor_copy(touch_d, xh[h][:, 0:1])
                dd = d_p.tile([P, d // 2], bf16, name="dd")
                nc.vector.scalar_tensor_tensor(
                    dd, gh[h], 0.0, xh[h], ALU.bypass, ALU.subtract
                )  # waits gh-DMA
                tr3 = trash_p.tile([P, d // 2], bf16, name="tr")
                nc.vector.scalar_tensor_tensor(
                    tr3, dd, w2[:, t : t + 1], dd, ALU.mult, ALU.mult,
                    accum_out=cols[:, NT - 1 + h : NT + h],
                )

            # ACT-issued store: after sq(6) in ACT program order; its only
            # foreign wait is DVE's final accumulate.
            nc.scalar.dma_start(out=out[:, 0:NCOLS], in_=cols)
    return nc


def _get_nc():
    if "nc" not in _CACHE:
        nc = build()
        nc.finalize()  # runs Bacc's passes (event-sem wait splitting, regalloc)
        _CACHE["nc"] = nc
    return _CACHE["nc"]


def _install_profile_hook():
    """Register the NTFF profile hook that this container's stripped antenv
    lacks: a ctypes bridge into libaxon_pjrt.so (same ABI trn_boot.py uses).
    Only needed for trace=True runs."""
    if "antenv.axon_hooks" in sys.modules:
        return
    import contextlib
    import ctypes
    import types

    so_path = "/opt/axon/libaxon_pjrt.so"
    lib = ctypes.CDLL(so_path)
    if not hasattr(lib, "axon_start_nrt_profile"):
        return
    lib.axon_start_nrt_profile.argtypes = [
        ctypes.POINTER(ctypes.c_int64),
        ctypes.c_size_t,
    ]
    lib.axon_start_nrt_profile.restype = ctypes.c_int64
    lib.axon_stop_nrt_profile.argtypes = [ctypes.c_char_p]
    lib.axon_stop_nrt_profile.restype = ctypes.c_int64

    @contextlib.contextmanager
    def _hook(output_dir, device_ids):
        import jax

        jax.devices()
        if device_ids:
            ids = (ctypes.c_int64 * len(device_ids))(*device_ids)
            rc = lib.axon_start_nrt_profile(ids, len(device_ids))
        else:
            rc = lib.axon_start_nrt_profile(None, 0)
        if rc != 0:
            raise RuntimeError(f"axon_start_nrt_profile rc={rc}")
        try:
            yield
        finally:
            n = lib.axon_stop_nrt_profile(str(output_dir).encode())
            print(f"profile: {n} file(s) written to {output_dir}")

    mod = types.ModuleType("antenv.axon_hooks")
    mod.get_axon_ntff_profile_hook = lambda: _hook
    sys.modules["antenv.axon_hooks"] = mod


def kernel(**inputs):
    from concourse.bass_utils import run_bass_kernel_spmd

    nc = _get_nc()
    names = ["input", "target", "weight", "sub_target", "target_pre", "sub_obrT"]
    arrs = {k: np.ascontiguousarray(np.asarray(inputs[k], dtype=np.float32)) for k in names}
    in_maps = []
    for c in range(NCORES):
        sl = slice(c * ROWS, (c + 1) * ROWS)
        m = {k: np.ascontiguousarray(v[sl]) for k, v in arrs.items()}
        # wcols[p, t] = w[t*128 + p] for this core's row slice (+ square):
        # a 1KB host transform that replaces an on-device scatter gather.
        wc = np.ascontiguousarray(arrs["weight"][sl].reshape(NT, P).T)
        m["wcols"] = wc
        m["w2cols"] = np.ascontiguousarray(wc * wc)
        in_maps.append(m)

    trace = os.environ.get("BASS_KERNEL_PROFILE", "0") == "1"
    if trace:
        _install_profile_hook()
    res = run_bass_kernel_spmd(nc, in_maps, list(range(NCORES)), trace=trace)

    mse_sum = 0.0
    cl_sum = 0.0
    for r in res.results:
        part = np.asarray(r["partials"], dtype=np.float64)
        mse_sum += part[:, :MSE_COLS].sum()
        cl_sum -= part[:, MSE_COLS:].sum()  # bce*ob <= 0: |.| = -(.)
    tot = float(N) * float(D)
    if trace and res.exec_time_ns is not None:
        print(f"HW exec time: {res.exec_time_ns} ns")
    return (
        np.asarray(np.float32(mse_sum / tot)),
        np.asarray(np.float32(cl_sum / tot)),
    )


# revision 19
# speedup vs baseline: 1.0036x; 1.0036x over previous
"""Trainium2 Bass kernel for nn_Loss_9749575762182.

Computes two scalar losses over (8192, 2048) fp32 tensors:
  wmse = mean((weight[:,None] * (target - input))**2)
  wcl  = mean(|(st*ln(tp+eps) + (1-st)*ln(1-tp+eps)) * obrT|)

Strategy: data-parallel over the row axis across 8 NeuronCores
(1024 rows each). Each core streams its 5 x 8MB tensor slices through
SBUF in eight [128, 2048] tiles, producing per-partition partial sums;
the tiny [128, 20] partials land back in DRAM and the host finishes
the reduction in float64.

The kernel is HBM-bound: 40MB/core over ~358GB/s peak = 112us floor;
~330GB/s (121us) is the observed practical stream rate. Evolution:
  v1 155.7us: 8 tiles, coarse tail (a full-2048 serial chain after the
      last byte) cost ~24us past the end of streaming.
  v2 143.1us: 18 column-split chunks (1024/512 wide). Short tail, but
      2-4KB DMA descriptors + 2x the dispatches/semaphores slowed the
      stream by ~4us and added ~1us stalls at chunk seams.
  v3: loads stay full [128, 2048] tiles (1MB contiguous, 8KB descriptor
      rows = best DMA efficiency, 40 dispatches total), while COMPUTE
      on the last tile is split into 1024+512+512 column passes so the
      post-stream serial chain is ~512-wide (~4us). Other wins kept:
      - First ACT instruction is an Ln touch: Bacc's
        insert_act_table_loads picks act-func-set 5 (ln+square+abs+copy)
        once, instead of set 0 + a 1283ns reload at the first real Ln.
      - mse/cl partials accumulate into ONE [128, 20] tile; the final
        store is issued from the ACT engine right after the last accum
        (ACT program order: no cross-engine semaphore, HWDGE latency).
      - Loads issue q (target_pre) first so the ACT ln chain starts
        as early as possible; per-tile compute runs l1/l2 first.

Per tile the math keeps the Vector and Scalar engines well under the
~14.8us/tile DMA budget:
  ACT: l1 = Ln(tp + eps)          (bias/scale fold the affine into the LUT)
  ACT: l2 = Ln(-tp + (1+eps))
  DVE: diff = target - input                     (tensor_tensor sub)
  ACT: Square(diff * w)  + accum -> mse partial  (scale = per-partition w)
  DVE: d = l1 - l2 ; m = st * d ; b = m + l2 ; po = b * obrT
  ACT: Abs(po) + accum -> cl partial

Hard-won environment notes (axon-tunneled trn2, this toolchain):
  - Build on bacc.Bacc() and call nc.finalize() before run_bass_via_pjrt;
    raw bass.Bass() BIR fails walrus ("Reg has not been allocated"), and
    without Bacc's generate_event_semaphores pass any instruction with
    >1 semaphore wait dies in codegen ("Too many sync wait commands").
  - tensor_tensor_reduce compiles + simulates fine but faults on real HW
    via the PJRT path; ACT Abs with accum_out replaces it.
  - Big loads go through nc.sync.dma_start (HW-DGE, fans out across HW
    queues): all-gpsimd SWDGE funnels through ONE dynamic queue
    (~216 GB/s ceiling observed -> 185us); HW-DGE gets 153us.
  - The CoreV3 ISA allows one sync-wait per instruction, and Tile
    doesn't split excess waits for free. Discipline: every instruction
    may depend on at most ONE foreign semaphore; tiny "touch" ops
    consume extra waits so the real consumers inherit them via engine
    program order / already-observed clocks.
"""

import os
import sys

if "/opt/trn_rl_repo" not in sys.path:
    sys.path.insert(0, "/opt/trn_rl_repo")

import numpy as np

N, D = 8192, 2048
NCORES = 8
ROWS = N // NCORES  # rows per core
P = 128             # SBUF partitions
NT = ROWS // P      # row-blocks per core (8)
EPS = 1e-10

# accumulator columns: 9 mse (7 full tiles via ACT Square + 2 DVE tail
# halves), 16 cl (two per tile: sum(c1*l1), sum(c2*l2)).
MSE_COLS = NT + 1
CL_COLS = 2 * NT
NCOLS = MSE_COLS + CL_COLS

_CACHE = {}


def build(rows=ROWS, d=D):
    import concourse.bacc as bacc
    import concourse.tile as tile
    from concourse import mybir

    f32 = mybir.dt.float32
    bf16 = mybir.dt.bfloat16
    ACTF = mybir.ActivationFunctionType
    ALU = mybir.AluOpType

    nc = bacc.Bacc()
    inp = nc.dram_tensor("input", [rows, d], f32, kind="ExternalInput")
    tgt = nc.dram_tensor("target", [rows, d], f32, kind="ExternalInput")
    wgt = nc.dram_tensor("weight", [rows], f32, kind="ExternalInput")
    st = nc.dram_tensor("sub_target", [rows, d], f32, kind="ExternalInput")
    tp = nc.dram_tensor("target_pre", [rows, d], f32, kind="ExternalInput")
    ob = nc.dram_tensor("sub_obrT", [rows, d], f32, kind="ExternalInput")
    # host-transposed weight columns: wcols[p, t] = w[t*128 + p], plus the
    # square. Contiguous [128, NT] HWDGE loads replace a 1024-descriptor
    # SWDGE gather that serialized ~17us onto one DMA queue (the stream
    # straggler in every earlier version of this kernel).
    wcols_d = nc.dram_tensor("wcols", [P, NT], f32, kind="ExternalInput")
    w2cols_d = nc.dram_tensor("w2cols", [P, NT], f32, kind="ExternalInput")
    out = nc.dram_tensor("partials", [P, NCOLS], f32, kind="ExternalOutput")

    inp_t = inp.rearrange("(t p) d -> t p d", p=P)
    tgt_t = tgt.rearrange("(t p) d -> t p d", p=P)
    st_t = st.rearrange("(t p) d -> t p d", p=P)
    tp_t = tp.rearrange("(t p) d -> t p d", p=P)
    ob_t = ob.rearrange("(t p) d -> t p d", p=P)

    with tile.TileContext(nc) as tc:
        with (
            tc.tile_pool(name="singles", bufs=1) as singles,
            tc.tile_pool(name="in_p", bufs=3) as in_p,
            tc.tile_pool(name="tgt_p", bufs=3) as tgt_p,
            tc.tile_pool(name="st_p", bufs=4) as st_p,
            tc.tile_pool(name="tp_p", bufs=3) as tp_p,
            tc.tile_pool(name="ob_p", bufs=4) as ob_p,
            tc.tile_pool(name="l1_p", bufs=2) as l1_p,
            tc.tile_pool(name="l2_p", bufs=2) as l2_p,
            tc.tile_pool(name="d_p", bufs=2) as d_p,
            tc.tile_pool(name="c1_p", bufs=2) as c1_p,
            tc.tile_pool(name="c2_p", bufs=2) as c2_p,
            tc.tile_pool(name="trash_p", bufs=1) as trash_p,
            tc.tile_pool(name="sqd_p", bufs=1) as sqd_p,
        ):
            w_cols = singles.tile([P, NT], f32)
            nc.scalar.dma_start(out=w_cols, in_=wcols_d[:, 0:NT])
            w2 = singles.tile([P, NT], f32)
            nc.scalar.dma_start(out=w2, in_=w2cols_d[:, 0:NT])
            cols = singles.tile([P, NCOLS], f32)
            eps_b = singles.tile([P, 1], f32)
            nc.vector.memset(eps_b, EPS)
            one_eps_b = singles.tile([P, 1], f32)
            nc.vector.memset(one_eps_b, 1.0 + EPS)
            zero_b = singles.tile([P, 1], f32)
            nc.vector.memset(zero_b, 0.0)

            gate_b = singles.tile([P, 1], f32)
            touch_d = singles.tile([P, 1], f32)
            atouch_d = singles.tile([P, 1], f32)
            # First ACT instruction is an Ln: loads act-func-set 5 once and
            # consumes the DVE-memset wait; the next two consume the
            # wcols/w2 DMA-completion waits.
            nc.scalar.activation(
                out=atouch_d, in_=zero_b, func=ACTF.Ln, bias=zero_b, scale=1.0
            )
            nc.scalar.activation(
                out=atouch_d, in_=w_cols[:, 0:1], func=ACTF.Ln, bias=zero_b, scale=1.0
            )
            nc.scalar.activation(
                out=atouch_d, in_=w2[:, 0:1], func=ACTF.Ln, bias=zero_b, scale=1.0
            )

            # sq(t) is deferred one tile and pinned after the next tile's
            # Lns via the gate bias (a zero that data-depends on l2), so
            # DVE's cl-accums never queue behind it on ACT.
            pending_sq = None

            def emit_sq(p, l2gate):
                ddp, wc, col = p
                nc.scalar.activation(
                    out=gate_b, in_=l2gate[:, 0:1], func=ACTF.Copy, scale=0.0
                )
                sq = sqd_p.tile([P, d], bf16, name="sq")
                nc.scalar.activation(
                    out=sq, in_=ddp, func=ACTF.Square, bias=gate_b,
                    scale=wc, accum_out=col,
                )  # waits PE (dd in PSUM)

            # ---- tiles 0..6: loads q,s,o,x,g ([128,2048], 8KB descriptor
            # rows = line-rate DMA). PE computes dd = g - x into PSUM; ACT:
            # l1, l2, Square(prev); DVE: cc1, cc2 and the two cl accums.
            for t in range(NT - 1):
                q = tp_p.tile([P, d], f32, name="q")
                nc.sync.dma_start(out=q, in_=tp_t[t])
                s = st_p.tile([P, d], f32, name="s")
                nc.sync.dma_start(out=s, in_=st_t[t])
                o = ob_p.tile([P, d], f32, name="o")
                nc.sync.dma_start(out=o, in_=ob_t[t])
                x = in_p.tile([P, d], f32, name="x")
                nc.sync.dma_start(out=x, in_=inp_t[t])
                g = tgt_p.tile([P, d], f32, name="g")
                nc.sync.dma_start(out=g, in_=tgt_t[t])

                l1 = l1_p.tile([P, d], bf16, name="l1")
                nc.scalar.activation(out=l1, in_=q, func=ACTF.Ln, bias=eps_b, scale=1.0)
                l2 = l2_p.tile([P, d], bf16, name="l2")
                nc.scalar.activation(
                    out=l2, in_=q, func=ACTF.Ln, bias=one_eps_b, scale=-1.0
                )
                if pending_sq is not None:
                    emit_sq(pending_sq, l2)

                nc.vector.tensor_copy(touch_d, x[:, 0:1])  # consume x-DMA wait
                dd = d_p.tile([P, d], bf16, name="dd")
                nc.vector.scalar_tensor_tensor(
                    dd, g, 0.0, x, ALU.bypass, ALU.subtract
                )  # waits g-DMA
                pending_sq = (dd, w_cols[:, t : t + 1], cols[:, t : t + 1])

                nc.vector.tensor_copy(touch_d, s[:, 0:1])  # consume s-DMA wait
                cc1 = c1_p.tile([P, d], bf16, name="cc1")
                nc.vector.scalar_tensor_tensor(
                    cc1, s, 0.0, o, ALU.bypass, ALU.mult
                )  # waits o-DMA
                cc2 = c2_p.tile([P, d], bf16, name="cc2")
                nc.vector.scalar_tensor_tensor(cc2, o, 0.0, cc1, ALU.bypass, ALU.subtract)
                nc.vector.tensor_copy(touch_d, l2[:, 0:1])  # consume ACT-l2 wait
                tr = trash_p.tile([P, d], bf16, name="tr")
                nc.vector.scalar_tensor_tensor(
                    tr, cc1, 0.0, l1, ALU.bypass, ALU.mult,
                    accum_out=cols[:, MSE_COLS + 2 * t : MSE_COLS + 2 * t + 1],
                )
                tr2 = trash_p.tile([P, d], bf16, name="tr")
                nc.vector.scalar_tensor_tensor(
                    tr2, cc2, 0.0, l2, ALU.bypass, ALU.mult,
                    accum_out=cols[:, MSE_COLS + 2 * t + 1 : MSE_COLS + 2 * t + 2],
                )

            # ---- tile 7: loads s,o,q then x,g in halves; compute all-DVE
            # (dd via STT with w^2) so the post-stream chain is just
            # dd1 + mse1 + the ACT-issued store.
            t = NT - 1
            s = st_p.tile([P, d], f32, name="s")
            nc.sync.dma_start(out=s, in_=st_t[t])
            o = ob_p.tile([P, d], f32, name="o")
            nc.sync.dma_start(out=o, in_=ob_t[t])
            q = tp_p.tile([P, d], f32, name="q")
            nc.sync.dma_start(out=q, in_=tp_t[t])
            xh, gh = [], []
            for h in range(2):
                c0, c1 = h * (d // 2), (h + 1) * (d // 2)
                xk = in_p.tile([P, d // 2], f32, name="x")
                nc.sync.dma_start(out=xk, in_=inp_t[t][:, c0:c1])
                gk = tgt_p.tile([P, d // 2], f32, name="g")
                nc.sync.dma_start(out=gk, in_=tgt_t[t][:, c0:c1])
                xh.append(xk)
                gh.append(gk)

            l1 = l1_p.tile([P, d], bf16, name="l1")
            nc.scalar.activation(out=l1, in_=q, func=ACTF.Ln, bias=eps_b, scale=1.0)
            l2 = l2_p.tile([P, d], bf16, name="l2")
            nc.scalar.activation(
                out=l2, in_=q, func=ACTF.Ln, bias=one_eps_b, scale=-1.0
            )
            emit_sq(pending_sq, l2)  # sq(6), after l2(7) in ACT order

            nc.vector.tensor_copy(touch_d, s[:, 0:1])  # consume s-DMA wait
            cc1 = c1_p.tile([P, d], bf16, name="cc1")
            nc.vector.scalar_tensor_tensor(
                cc1, s, 0.0, o, ALU.bypass, ALU.mult
            )  # waits o-DMA
            cc2 = c2_p.tile([P, d], bf16, name="cc2")
            nc.vector.scalar_tensor_tensor(cc2, o, 0.0, cc1, ALU.bypass, ALU.subtract)
            nc.vector.tensor_copy(touch_d, l2[:, 0:1])  # consume ACT-l2 wait
            tr = trash_p.tile([P, d], bf16, name="tr")
            nc.vector.scalar_tensor_tensor(
                tr, cc1, 0.0, l1, ALU.bypass, ALU.mult,
                accum_out=cols[:, MSE_COLS + 2 * t : MSE_COLS + 2 * t + 1],
            )
            tr2 = trash_p.tile([P, d], bf16, name="tr")
            nc.vector.scalar_tensor_tensor(
                tr2, cc2, 0.0, l2, ALU.bypass, ALU.mult,
                accum_out=cols[:, MSE_COLS + 2 * t + 1 : MSE_COLS + 2 * t + 2],
            )
            for h in range(2):
                nc.vector.tensor_copy(touch_d, xh[h][:, 0:1])
                dd = d_p.tile([P, d // 2], bf16, name="dd")
                nc.vector.scalar_tensor_tensor(
                    dd, gh[h], 0.0, xh[h], ALU.bypass, ALU.subtract
                )  # waits gh-DMA
                tr3 = trash_p.tile([P, d // 2], bf16, name="tr")
                nc.vector.scalar_tensor_tensor(
                    tr3, dd, w2[:, t : t + 1], dd, ALU.mult, ALU.mult,
                    accum_out=cols[:, NT - 1 + h : NT + h],
                )

            # ACT-issued store: after sq(6) in ACT program order; its only
            # foreign wait is DVE's final accumulate.
            nc.scalar.dma_start(out=out[:, 0:NCOLS], in_=cols)
    return nc


def _get_nc():
    if "nc" not in _CACHE:
        nc = build()
        nc.finalize()  # runs Bacc's passes (event-sem wait splitting, regalloc)
        _CACHE["nc"] = nc
    return _CACHE["nc"]


def _install_profile_hook():
    """Register the NTFF profile hook that this container's stripped antenv
    lacks: a ctypes bridge into libaxon_pjrt.so (same ABI trn_boot.py uses).
    Only needed for trace=True runs."""
    if "antenv.axon_hooks" in sys.modules:
        return
    import contextlib
    import ctypes
    import types

    so_path = "/opt/axon/libaxon_pjrt.so"
    lib = ctypes.CDLL(so_path)
    if not hasattr(lib, "axon_start_nrt_profile"):
        return
    lib.axon_start_nrt_profile.argtypes = [
        ctypes.POINTER(ctypes.c_int64),
        ctypes.c_size_t,
    ]
    lib.axon_start_nrt_profile.restype = ctypes.c_int64
    lib.axon_stop_nrt_profile.argtypes = [ctypes.c_char_p]
    lib.axon_stop_nrt_profile.restype = ctypes.c_int64

    @contextlib.contextmanager
    def _hook(output_dir, device_ids):
        import jax

        jax.devices()
        if device_ids:
            ids = (ctypes.c_int64 * len(device_ids))(*device_ids)
            rc = lib.axon_start_nrt_profile(ids, len(device_ids))
        else:
            rc = lib.axon_start_nrt_profile(None, 0)
        if rc != 0:
            raise RuntimeError(f"axon_start_nrt_profile rc={rc}")
        try:
            yield
        finally:
            n = lib.axon_stop_nrt_profile(str(output_dir).encode())
            print(f"profile: {n} file(s) written to {output_dir}")

    mod = types.ModuleType("antenv.axon_hooks")
    mod.get_axon_ntff_profile_hook = lambda: _hook
    sys.modules["antenv.axon_hooks"] = mod


def kernel(**inputs):
    from concourse.bass_utils import run_bass_kernel_spmd

    nc = _get_nc()
    names = ["input", "target", "weight", "sub_target", "target_pre", "sub_obrT"]
    arrs = {k: np.ascontiguousarray(np.asarray(inputs[k], dtype=np.float32)) for k in names}
    in_maps = []
    for c in range(NCORES):
        sl = slice(c * ROWS, (c + 1) * ROWS)
        m = {k: np.ascontiguousarray(v[sl]) for k, v in arrs.items()}
        # wcols[p, t] = w[t*128 + p] for this core's row slice (+ square):
        # a 1KB host transform that replaces an on-device scatter gather.
        wc = np.ascontiguousarray(arrs["weight"][sl].reshape(NT, P).T)
        m["wcols"] = wc
        m["w2cols"] = np.ascontiguousarray(wc * wc)
        in_maps.append(m)

    trace = os.environ.get("BASS_KERNEL_PROFILE", "0") == "1"
    if trace:
        _install_profile_hook()
    res = run_bass_kernel_spmd(nc, in_maps, list(range(NCORES)), trace=trace)

    mse_sum = 0.0
    cl_sum = 0.0
    for r in res.results:
        part = np.asarray(r["partials"], dtype=np.float64)
        mse_sum += part[:, :MSE_COLS].sum()
        cl_sum -= part[:, MSE_COLS:].sum()  # bce*ob <= 0: |.| = -(.)
    tot = float(N) * float(D)
    if trace and res.exec_time_ns is not None:
        print(f"HW exec time: {res.exec_time_ns} ns")
    return (
        np.asarray(np.float32(mse_sum / tot)),
        np.asarray(np.float32(cl_sum / tot)),
    )


# revision 20
# speedup vs baseline: 1.0826x; 1.0788x over previous
"""Trainium2 Bass kernel for nn_Loss_9749575762182.

Computes two scalar losses over (8192, 2048) fp32 tensors:
  wmse = mean((weight[:,None] * (target - input))**2)
  wcl  = mean(|(st*ln(tp+eps) + (1-st)*ln(1-tp+eps)) * obrT|)

Strategy: data-parallel over the row axis across 8 NeuronCores
(1024 rows each). Each core streams its 5 x 8MB tensor slices through
SBUF in eight [128, 2048] tiles, producing per-partition partial sums;
the tiny [128, 20] partials land back in DRAM and the host finishes
the reduction in float64.

The kernel is HBM-bound: 40MB/core over ~358GB/s peak = 112us floor;
~330GB/s (121us) is the observed practical stream rate. Evolution:
  v1 155.7us: 8 tiles, coarse tail (a full-2048 serial chain after the
      last byte) cost ~24us past the end of streaming.
  v2 143.1us: 18 column-split chunks (1024/512 wide). Short tail, but
      2-4KB DMA descriptors + 2x the dispatches/semaphores slowed the
      stream by ~4us and added ~1us stalls at chunk seams.
  v3: loads stay full [128, 2048] tiles (1MB contiguous, 8KB descriptor
      rows = best DMA efficiency, 40 dispatches total), while COMPUTE
      on the last tile is split into 1024+512+512 column passes so the
      post-stream serial chain is ~512-wide (~4us). Other wins kept:
      - First ACT instruction is an Ln touch: Bacc's
        insert_act_table_loads picks act-func-set 5 (ln+square+abs+copy)
        once, instead of set 0 + a 1283ns reload at the first real Ln.
      - mse/cl partials accumulate into ONE [128, 20] tile; the final
        store is issued from the ACT engine right after the last accum
        (ACT program order: no cross-engine semaphore, HWDGE latency).
      - Loads issue q (target_pre) first so the ACT ln chain starts
        as early as possible; per-tile compute runs l1/l2 first.

Per tile the math keeps the Vector and Scalar engines well under the
~14.8us/tile DMA budget:
  ACT: l1 = Ln(tp + eps)          (bias/scale fold the affine into the LUT)
  ACT: l2 = Ln(-tp + (1+eps))
  DVE: diff = target - input                     (tensor_tensor sub)
  ACT: Square(diff * w)  + accum -> mse partial  (scale = per-partition w)
  DVE: d = l1 - l2 ; m = st * d ; b = m + l2 ; po = b * obrT
  ACT: Abs(po) + accum -> cl partial

Hard-won environment notes (axon-tunneled trn2, this toolchain):
  - Build on bacc.Bacc() and call nc.finalize() before run_bass_via_pjrt;
    raw bass.Bass() BIR fails walrus ("Reg has not been allocated"), and
    without Bacc's generate_event_semaphores pass any instruction with
    >1 semaphore wait dies in codegen ("Too many sync wait commands").
  - tensor_tensor_reduce compiles + simulates fine but faults on real HW
    via the PJRT path; ACT Abs with accum_out replaces it.
  - Big loads go through nc.sync.dma_start (HW-DGE, fans out across HW
    queues): all-gpsimd SWDGE funnels through ONE dynamic queue
    (~216 GB/s ceiling observed -> 185us); HW-DGE gets 153us.
  - The CoreV3 ISA allows one sync-wait per instruction, and Tile
    doesn't split excess waits for free. Discipline: every instruction
    may depend on at most ONE foreign semaphore; tiny "touch" ops
    consume extra waits so the real consumers inherit them via engine
    program order / already-observed clocks.
"""

import os
import sys

if "/opt/trn_rl_repo" not in sys.path:
    sys.path.insert(0, "/opt/trn_rl_repo")

import numpy as np

N, D = 8192, 2048
NCORES = 8
ROWS = N // NCORES  # rows per core
P = 128             # SBUF partitions
NT = ROWS // P      # row-blocks per core (8)
EPS = 1e-10

# accumulator columns: 9 mse (7 full tiles + 2 tail halves), 16 cl
# (two per tile: sum(c1*l1), sum(c2*l2)); all accumulated on DVE.
MSE_COLS = NT + 1
CL_COLS = 2 * NT
NCOLS = MSE_COLS + CL_COLS

_CACHE = {}


def build(rows=ROWS, d=D):
    import concourse.bacc as bacc
    import concourse.tile as tile
    from concourse import mybir

    f32 = mybir.dt.float32
    ACTF = mybir.ActivationFunctionType
    ALU = mybir.AluOpType

    nc = bacc.Bacc()
    inp = nc.dram_tensor("input", [rows, d], f32, kind="ExternalInput")
    tgt = nc.dram_tensor("target", [rows, d], f32, kind="ExternalInput")
    wgt = nc.dram_tensor("weight", [rows], f32, kind="ExternalInput")
    st = nc.dram_tensor("sub_target", [rows, d], f32, kind="ExternalInput")
    tp = nc.dram_tensor("target_pre", [rows, d], f32, kind="ExternalInput")
    ob = nc.dram_tensor("sub_obrT", [rows, d], f32, kind="ExternalInput")
    # host-squared, host-transposed weight columns: w2cols[p, t] =
    # w[t*128 + p]**2. One contiguous [128, NT] HWDGE load replaces a
    # 1024-descriptor SWDGE gather plus an on-device square.
    w2cols_d = nc.dram_tensor("w2cols", [P, NT], f32, kind="ExternalInput")
    out = nc.dram_tensor("partials", [P, NCOLS], f32, kind="ExternalOutput")

    inp_t = inp.rearrange("(t p) d -> t p d", p=P)
    tgt_t = tgt.rearrange("(t p) d -> t p d", p=P)
    st_t = st.rearrange("(t p) d -> t p d", p=P)
    tp_t = tp.rearrange("(t p) d -> t p d", p=P)
    ob_t = ob.rearrange("(t p) d -> t p d", p=P)

    with tile.TileContext(nc) as tc:
        with (
            tc.tile_pool(name="singles", bufs=1) as singles,
            tc.tile_pool(name="in_p", bufs=2) as in_p,
            tc.tile_pool(name="tgt_p", bufs=2) as tgt_p,
            tc.tile_pool(name="st_p", bufs=2) as st_p,
            tc.tile_pool(name="tp_p", bufs=2) as tp_p,
            tc.tile_pool(name="ob_p", bufs=3) as ob_p,
            tc.tile_pool(name="l1_p", bufs=2) as l1_p,
            tc.tile_pool(name="l2_p", bufs=2) as l2_p,
            tc.tile_pool(name="d_p", bufs=2) as d_p,
            tc.tile_pool(name="c1_p", bufs=2) as c1_p,
            tc.tile_pool(name="c2_p", bufs=2) as c2_p,
            tc.tile_pool(name="trash_p", bufs=1) as trash_p,
        ):
            w2 = singles.tile([P, NT], f32)
            nc.scalar.dma_start(out=w2, in_=w2cols_d[:, 0:NT])
            # per-partition accumulator columns, all written by DVE
            # accum_outs -> single in-order writer; SP stores at the end.
            cols = singles.tile([P, NCOLS], f32)
            eps_b = singles.tile([P, 1], f32)
            nc.vector.memset(eps_b, EPS)
            one_eps_b = singles.tile([P, 1], f32)
            nc.vector.memset(one_eps_b, 1.0 + EPS)
            zero_b = singles.tile([P, 1], f32)
            nc.vector.memset(zero_b, 0.0)

            touch_d = singles.tile([P, 1], f32)
            atouch_d = singles.tile([P, 1], f32)
            # First ACT instruction is an Ln: loads act-func-set 5 once and
            # consumes the DVE-memset wait (zero_b is the last memset).
            nc.scalar.activation(
                out=atouch_d, in_=zero_b, func=ACTF.Ln, bias=zero_b, scale=1.0
            )

            mse_c = 0
            cl_c = MSE_COLS

            def lns(q):
                l1 = l1_p.tile([P, d], bf16=False, name="l1") if False else l1_p.tile([P, d], f32, name="l1")
                nc.scalar.activation(out=l1, in_=q, func=ACTF.Ln, bias=eps_b, scale=1.0)
                l2 = l2_p.tile([P, d], f32, name="l2")
                nc.scalar.activation(
                    out=l2, in_=q, func=ACTF.Ln, bias=one_eps_b, scale=-1.0
                )
                return l1, l2

            def mse_pass(x, g, wc, cw):
                nonlocal mse_c
                nc.vector.tensor_copy(touch_d, x[:, 0:1])  # consume x-DMA wait
                dd = d_p.tile([P, cw], f32, name="dd")
                nc.vector.scalar_tensor_tensor(
                    dd, g, 0.0, x, ALU.bypass, ALU.subtract
                )  # waits g-DMA
                tr = trash_p.tile([P, cw], f32, name="tr")
                nc.vector.scalar_tensor_tensor(
                    tr, dd, wc, dd, ALU.mult, ALU.mult,
                    accum_out=cols[:, mse_c : mse_c + 1],
                )
                mse_c += 1

            def cl_pass(s, o, l1, l2):
                nonlocal cl_c
                nc.vector.tensor_copy(touch_d, s[:, 0:1])  # consume s-DMA wait
                cc1 = c1_p.tile([P, d], f32, name="cc1")
                nc.vector.scalar_tensor_tensor(
                    cc1, s, 0.0, o, ALU.bypass, ALU.mult
                )  # waits o-DMA
                cc2 = c2_p.tile([P, d], f32, name="cc2")
                nc.vector.scalar_tensor_tensor(cc2, o, 0.0, cc1, ALU.bypass, ALU.subtract)
                nc.vector.tensor_copy(touch_d, l2[:, 0:1])  # consume ACT-l2 wait
                tr = trash_p.tile([P, d], f32, name="tr")
                nc.vector.scalar_tensor_tensor(
                    tr, cc1, 0.0, l1, ALU.bypass, ALU.mult,
                    accum_out=cols[:, cl_c : cl_c + 1],
                )
                tr2 = trash_p.tile([P, d], f32, name="tr")
                nc.vector.scalar_tensor_tensor(
                    tr2, cc2, 0.0, l2, ALU.bypass, ALU.mult,
                    accum_out=cols[:, cl_c + 1 : cl_c + 2],
                )
                cl_c += 2

            # ---- tiles 0..6: full-width single pass, all compute on DVE
            # (ACT only runs the two Lns): one-directional dependency flow,
            # minimal instruction/semaphore count.
            for t in range(NT - 1):
                q = tp_p.tile([P, d], f32, name="q")
                nc.sync.dma_start(out=q, in_=tp_t[t])
                x = in_p.tile([P, d], f32, name="x")
                nc.sync.dma_start(out=x, in_=inp_t[t])
                g = tgt_p.tile([P, d], f32, name="g")
                nc.sync.dma_start(out=g, in_=tgt_t[t])
                s = st_p.tile([P, d], f32, name="s")
                nc.sync.dma_start(out=s, in_=st_t[t])
                o = ob_p.tile([P, d], f32, name="o")
                nc.sync.dma_start(out=o, in_=ob_t[t])

                l1, l2 = lns(q)
                mse_pass(x, g, w2[:, t : t + 1], d)
                cl_pass(s, o, l1, l2)

            # ---- tile 7: loads reordered (s,o,q first; x,g split in half)
            # so the post-stream chain is dd1 + mse1 + the store.
            t = NT - 1
            s = st_p.tile([P, d], f32, name="s")
            nc.sync.dma_start(out=s, in_=st_t[t])
            o = ob_p.tile([P, d], f32, name="o")
            nc.sync.dma_start(out=o, in_=ob_t[t])
            q = tp_p.tile([P, d], f32, name="q")
            nc.sync.dma_start(out=q, in_=tp_t[t])
            xh, gh = [], []
            for h in range(2):
                c0, c1 = h * (d // 2), (h + 1) * (d // 2)
                xk = in_p.tile([P, d // 2], f32, name="x")
                nc.sync.dma_start(out=xk, in_=inp_t[t][:, c0:c1])
                gk = tgt_p.tile([P, d // 2], f32, name="g")
                nc.sync.dma_start(out=gk, in_=tgt_t[t][:, c0:c1])
                xh.append(xk)
                gh.append(gk)

            l1, l2 = lns(q)
            cl_pass(s, o, l1, l2)
            for h in range(2):
                mse_pass(xh[h], gh[h], w2[:, t : t + 1], d // 2)

            # SP-issued store: last in SP program order; cols has a single
            # writer engine (DVE), so one foreign wait.
            nc.sync.dma_start(out=out[:, 0:NCOLS], in_=cols)
    return nc


def _get_nc():
    if "nc" not in _CACHE:
        nc = build()
        nc.finalize()  # runs Bacc's passes (event-sem wait splitting, regalloc)
        _CACHE["nc"] = nc
    return _CACHE["nc"]


def _install_profile_hook():
    """Register the NTFF profile hook that this container's stripped antenv
    lacks: a ctypes bridge into libaxon_pjrt.so (same ABI trn_boot.py uses).
    Only needed for trace=True runs."""
    if "antenv.axon_hooks" in sys.modules:
        return
    import contextlib
    import ctypes
    import types

    so_path = "/opt/axon/libaxon_pjrt.so"
    lib = ctypes.CDLL(so_path)
    if not hasattr(lib, "axon_start_nrt_profile"):
        return
    lib.axon_start_nrt_profile.argtypes = [
        ctypes.POINTER(ctypes.c_int64),
        ctypes.c_size_t,
    ]
    lib.axon_start_nrt_profile.restype = ctypes.c_int64
    lib.axon_stop_nrt_profile.argtypes = [ctypes.c_char_p]
    lib.axon_stop_nrt_profile.restype = ctypes.c_int64

    @contextlib.contextmanager
    def _hook(output_dir, device_ids):
        import jax

        jax.devices()
        if device_ids:
            ids = (ctypes.c_int64 * len(device_ids))(*device_ids)
            rc = lib.axon_start_nrt_profile(ids, len(device_ids))
        else:
            rc = lib.axon_start_nrt_profile(None, 0)
        if rc != 0:
            raise RuntimeError(f"axon_start_nrt_profile rc={rc}")
        try:
            yield
        finally:
            n = lib.axon_stop_nrt_profile(str(output_dir).encode())
            print(f"profile: {n} file(s) written to {output_dir}")

    mod = types.ModuleType("antenv.axon_hooks")
    mod.get_axon_ntff_profile_hook = lambda: _hook
    sys.modules["antenv.axon_hooks"] = mod


def kernel(**inputs):
    from concourse.bass_utils import run_bass_kernel_spmd

    nc = _get_nc()
    names = ["input", "target", "weight", "sub_target", "target_pre", "sub_obrT"]
    arrs = {k: np.ascontiguousarray(np.asarray(inputs[k], dtype=np.float32)) for k in names}
    in_maps = []
    for c in range(NCORES):
        sl = slice(c * ROWS, (c + 1) * ROWS)
        m = {k: np.ascontiguousarray(v[sl]) for k, v in arrs.items()}
        # w2cols[p, t] = w[t*128 + p]**2 for this core's row slice: a 1KB
        # host transform replacing an on-device scatter gather + square.
        wc = arrs["weight"][sl].reshape(NT, P).T
        m["w2cols"] = np.ascontiguousarray(wc * wc)
        in_maps.append(m)

    trace = os.environ.get("BASS_KERNEL_PROFILE", "0") == "1"
    if trace:
        _install_profile_hook()
    res = run_bass_kernel_spmd(nc, in_maps, list(range(NCORES)), trace=trace)

    mse_sum = 0.0
    cl_sum = 0.0
    for r in res.results:
        part = np.asarray(r["partials"], dtype=np.float64)
        mse_sum += part[:, :MSE_COLS].sum()
        cl_sum -= part[:, MSE_COLS:].sum()  # bce*ob <= 0: |.| = -(.)
    tot = float(N) * float(D)
    if trace and res.exec_time_ns is not None:
        print(f"HW exec time: {res.exec_time_ns} ns")
    return (
        np.asarray(np.float32(mse_sum / tot)),
        np.asarray(np.float32(cl_sum / tot)),
    )
